# revision 9
# baseline (speedup 1.0000x reference)
"""NT-Xent contrastive loss on 8 Trainium2 NeuronCores.

Math (reference): Z = interleave(z1, z2) [2N, D]; Zn = row-normalize(Z);
S = exp(Zn @ Zn^T / T); loss = mean_i[ -log(S[i, i^1] / (rowsum_i - diag_i + 1e-8)) ].

Sharding: row-block parallel. Each core owns 2N/8 = 1024 rows of Z and computes
  rowsum_i  = sum_j exp(2 * zn_i . zn_j)   (full 8192-column sweep)
  s_pair_i  = zn_i . zn_{i^1}              (from the diagonal 128x128 sub-blocks)
  partial   = sum_i [ ln(rowsum_i - e^2 + 1e-8) - 2 * s_pair_i ]
The host sums the 8 partials and divides by 2N.  (diag_i = exp(2*||zn_i||^2) =
e^2 to ~1e-5 relative, and the denominator is ~8e3, so the constant is exact
far beyond the output tolerance.)

Layouts: the host ships Z^T (bf16, [D, 2N]) so both matmul operands are already
K-major; normalization happens on device: q_j = colsum(Z^T .^2) via a
ones-matmul (broadcast across partitions), rinv_j = exp(-0.5 * ln q_j) on the
scalar engine, then one elementwise multiply.  exp+rowsum are fused in one
scalar-engine pass per PSUM group via accum_out.
"""

import numpy as np
import ml_dtypes

N, D = 4096, 256
NC = 8                   # cores
RPC = 2 * N // NC        # rows of Z per core = 1024
MT = RPC // 128          # output m-tiles per core = 8
CB = 512                 # column block (one PSUM bank of fp32)
NCB = 2 * N // CB        # 16 column blocks
KC = D // 128            # 2 contraction chunks
GRP = 4                  # column blocks per PSUM group for the exp pass
NG = NCB // GRP
E2 = float(np.exp(2.0))

_prog_cache = {}


def _split_multi_waits(nc, maxw=1):
    """The walrus build in this container rejects instructions carrying more
    than one semaphore wait ("Too many sync wait commands").  Hoist extra
    waits onto single-wait NOPs inserted just before the instruction on the
    same engine stream — the engine sequencer processes waits in program
    order, so blocking semantics are identical."""
    import concourse.mybir as mybir

    n_split = 0
    n_nops = 0
    for f in nc.m.functions:
        for b in f.blocks:
            out = []
            dirty = False
            for ins in b.instructions:
                si = getattr(ins, "sync_info", None)
                ow = list(si.on_wait) if si is not None and si.on_wait else []
                if len(ow) > maxw:
                    extra, keep = ow[:-maxw], ow[-maxw:]
                    for w in extra:
                        nop = mybir.InstNoOp(
                            name=f"{ins.name}-wsplit{n_nops}", ins=[], outs=[])
                        nop.engine = ins.engine
                        nop.sync_info = mybir.SyncInfo(on_wait=[w], on_update=[])
                        out.append(nop)
                        n_nops += 1
                    ins.sync_info = mybir.SyncInfo(
                        on_wait=keep,
                        on_update=list(si.on_update) if si.on_update else [])
                    n_split += 1
                    dirty = True
                out.append(ins)
            if dirty:
                b.instructions = out
    return n_split, n_nops


def _build_program():
    import concourse.bass as bass
    import concourse.tile as tile
    import concourse.mybir as mybir

    f32 = mybir.dt.float32
    bf16 = mybir.dt.bfloat16
    AF = mybir.ActivationFunctionType
    OP = mybir.AluOpType
    X = mybir.AxisListType.X
    ts = bass.ts

    nc = bass.Bass("TRN2", name="ntxent")
    zt = nc.dram_tensor("zt", [D, 2 * N], bf16, kind="ExternalInput")
    ztb = nc.dram_tensor("ztb", [D, RPC], bf16, kind="ExternalInput")
    pmask = nc.dram_tensor("pmask", [128, 128], f32, kind="ExternalInput")
    partial = nc.dram_tensor("partial", [1, 1], f32, kind="ExternalOutput")

    with tile.TileContext(nc) as tc:
        with (
            tc.tile_pool(name="persist", bufs=1) as persist,
            tc.tile_pool(name="io", bufs=4) as io,
            tc.tile_pool(name="work", bufs=3) as work,
            tc.tile_pool(name="scr", bufs=2) as scr,
        ):
            cpps_cm = tc.tile_pool(name="cpps", bufs=2, space="PSUM")
            cpps = cpps_cm.__enter__()
            ones_bf = persist.tile([128, 128], bf16)
            nc.vector.memset(ones_bf, 1.0)
            ones_f = persist.tile([128, 1], f32)
            nc.vector.memset(ones_f, 1.0)
            pm = persist.tile([128, 128], f32)
            nc.sync.dma_start(pm, pmask[:, :])

            ztn = persist.tile([128, KC, 2 * N], bf16)   # normalized Z^T (rhs)
            ztnb = persist.tile([128, KC, RPC], bf16)    # normalized own block (lhsT)
            RS = persist.tile([128, MT], f32)            # rowsums
            SP = persist.tile([128, MT], f32)            # pair logits

            # ---- phase B: own block -> ztnb ----
            ztb_s = persist.tile([128, KC, RPC], bf16)
            for k in range(KC):
                nc.sync.dma_start(ztb_s[:, k, :], ztb[k * 128:(k + 1) * 128, :])
            sqb = scr.tile([128, KC, RPC], bf16, tag="sqb")
            for k in range(KC):
                nc.vector.tensor_mul(sqb[:, k, :], ztb_s[:, k, :], ztb_s[:, k, :])
            for cb in range(RPC // CB):
                qb = cpps.tile([128, CB], f32, tag="cp")
                for k in range(KC):
                    nc.tensor.matmul(qb, ones_bf, sqb[:, k, ts(cb, CB)],
                                     start=(k == 0), stop=(k == KC - 1))
                lnq = work.tile([128, CB], f32, tag="lnq")
                nc.scalar.activation(out=lnq, in_=qb, func=AF.Ln)
                rinv = work.tile([128, CB], bf16, tag="rinv")
                nc.scalar.activation(out=rinv, in_=lnq, func=AF.Exp, scale=-0.5)
                for k in range(KC):
                    nc.vector.tensor_mul(ztnb[:, k, ts(cb, CB)],
                                         ztb_s[:, k, ts(cb, CB)], rinv)

            # ---- phase P: pair logits from diagonal sub-blocks ----
            for m in range(MT):
                ssub = cpps.tile([128, 128], f32, tag="cp")
                for k in range(KC):
                    nc.tensor.matmul(ssub, ztnb[:, k, ts(m, 128)],
                                     ztnb[:, k, ts(m, 128)],
                                     start=(k == 0), stop=(k == KC - 1))
                junk = scr.tile([128, 128], f32, tag="junk")
                nc.vector.tensor_mul(junk, ssub, pm)
                nc.vector.reduce_sum(out=SP[:, m:m + 1], in_=junk, axis=X)

            # ---- phase C: full Z^T -> ztn, in 2048-col super-blocks ----
            SB = 4 * CB
            for sb in range(2 * N // SB):
                ztc = io.tile([128, KC, SB], bf16, tag="ztc")
                for k in range(KC):
                    nc.sync.dma_start(ztc[:, k, :],
                                      zt[k * 128:(k + 1) * 128, ts(sb, SB)])
                sqc = work.tile([128, KC, SB], bf16, tag="sqc")
                for k in range(KC):
                    nc.vector.tensor_mul(sqc[:, k, :], ztc[:, k, :], ztc[:, k, :])
                qc = cpps.tile([128, SB], f32, tag="cp")
                for ci in range(4):
                    for k in range(KC):
                        nc.tensor.matmul(qc[:, ts(ci, CB)], ones_bf,
                                         sqc[:, k, ts(ci, CB)],
                                         start=(k == 0), stop=(k == KC - 1))
                lnq = work.tile([128, SB], f32, tag="lnq")
                nc.scalar.activation(out=lnq, in_=qc, func=AF.Ln)
                rinv = work.tile([128, SB], bf16, tag="rinv")
                nc.scalar.activation(out=rinv, in_=lnq, func=AF.Exp, scale=-0.5)
                for k in range(KC):
                    nc.vector.tensor_mul(ztn[:, k, ts(sb, SB)], ztc[:, k, :], rinv)

            # Close the colsum PSUM pool so the main loop can use 4-bank
            # groups x2 buffers (all 8 banks).
            cpps_cm.__exit__(None, None, None)

            # ---- main: S-block matmuls + fused exp/rowsum ----
            with tc.tile_pool(name="mainps", bufs=2, space="PSUM") as mainps:
                for m in range(MT):
                    rs_m = scr.tile([128, NG], f32, tag="rsm")
                    for g in range(NG):
                        ps = mainps.tile([128, GRP * CB], f32, tag="main")
                        for ci in range(GRP):
                            cb = g * GRP + ci
                            for k in range(KC):
                                nc.tensor.matmul(ps[:, ts(ci, CB)],
                                                 ztnb[:, k, ts(m, 128)],
                                                 ztn[:, k, ts(cb, CB)],
                                                 start=(k == 0), stop=(k == KC - 1))
                        # exp in place (PSUM->PSUM): the exp'd matrix itself is
                        # discarded, only accum_out (the rowsum) is kept.
                        nc.scalar.activation(out=ps, in_=ps, func=AF.Exp,
                                             scale=2.0,
                                             accum_out=rs_m[:, g:g + 1])
                    nc.vector.reduce_sum(out=RS[:, m:m + 1], in_=rs_m, axis=X)

                # ---- final scalar ----
                DEN = persist.tile([128, MT], f32)
                nc.vector.tensor_scalar_add(DEN, RS, float(1e-8 - E2))
                LND = persist.tile([128, MT], f32)
                nc.scalar.activation(out=LND, in_=DEN, func=AF.Ln)
                LV = persist.tile([128, MT], f32)
                nc.vector.scalar_tensor_tensor(out=LV, in0=SP, scalar=-2.0,
                                               in1=LND, op0=OP.mult, op1=OP.add)
                fin = mainps.tile([1, MT], f32, tag="main")
                nc.tensor.matmul(fin, ones_f, LV, start=True, stop=True)
                tot = persist.tile([1, 1], f32)
                nc.vector.reduce_sum(out=tot, in_=fin, axis=X)
                nc.sync.dma_start(partial[:, :], tot)

    _split_multi_waits(nc)
    return nc


def _prepare_inputs(z1, z2):
    z1 = np.asarray(z1, dtype=np.float32)
    z2 = np.asarray(z2, dtype=np.float32)
    ztf = np.empty((D, 2 * N), dtype=np.float32)
    ztf[:, 0::2] = z1.T
    ztf[:, 1::2] = z2.T
    ztb16 = np.ascontiguousarray(ztf.astype(ml_dtypes.bfloat16))
    pmask = np.zeros((128, 128), dtype=np.float32)
    idx = np.arange(128)
    pmask[idx, idx ^ 1] = 1.0
    in_maps = []
    for c in range(NC):
        in_maps.append({
            "zt": ztb16,
            "ztb": np.ascontiguousarray(ztb16[:, c * RPC:(c + 1) * RPC]),
            "pmask": pmask,
        })
    return in_maps


def _run(z1, z2, trace=False):
    from concourse.bass_utils import run_bass_kernel_spmd
    if "nc" not in _prog_cache:
        _prog_cache["nc"] = _build_program()
    nc = _prog_cache["nc"]
    in_maps = _prepare_inputs(z1, z2)
    res = run_bass_kernel_spmd(nc, in_maps, core_ids=list(range(NC)), trace=trace)
    total = sum(float(r["partial"][0, 0]) for r in res.results)
    out = np.float32(total / (2 * N))
    return out, res


def kernel(z1, z2):
    out, _ = _run(z1, z2, trace=False)
    return out


# revision 11
# speedup vs baseline: 1.0034x; 1.0034x over previous
"""NT-Xent contrastive loss on 8 Trainium2 NeuronCores.

Math (reference): Z = interleave(z1, z2) [2N, D]; Zn = row-normalize(Z);
S = exp(Zn @ Zn^T / T); loss = mean_i[ -log(S[i, i^1] / (rowsum_i - diag_i + 1e-8)) ].

Sharding: row-block parallel. Each core owns 2N/8 = 1024 rows of Z and computes
  rowsum_i  = sum_j exp(2 * zn_i . zn_j)   (full 8192-column sweep)
  s_pair_i  = zn_i . zn_{i^1}              (from the diagonal 128x128 sub-blocks)
  partial   = sum_i [ ln(rowsum_i - e^2 + 1e-8) - 2 * s_pair_i ]
The host sums the 8 partials and divides by 2N.  (diag_i = exp(2*||zn_i||^2) =
e^2 to ~1e-5 relative, and the denominator is ~8e3, so the constant is exact
far beyond the output tolerance.)

Layouts: the host ships Z^T (bf16, [D, 2N]) so both matmul operands are already
K-major; normalization happens on device: q_j = colsum(Z^T .^2) via a
ones-matmul (broadcast across partitions), rinv_j = exp(-0.5 * ln q_j) on the
scalar engine, then one elementwise multiply.  exp+rowsum are fused in one
scalar-engine pass per PSUM group via accum_out.
"""

import numpy as np
import ml_dtypes

N, D = 4096, 256
NC = 8                   # cores
RPC = 2 * N // NC        # rows of Z per core = 1024
MT = RPC // 128          # output m-tiles per core = 8
CB = 512                 # column block (one PSUM bank of fp32)
NCB = 2 * N // CB        # 16 column blocks
KC = D // 128            # 2 contraction chunks
GRP = 4                  # column blocks per PSUM group for the exp pass
NG = NCB // GRP
E2 = float(np.exp(2.0))

_prog_cache = {}


def _split_multi_waits(nc, maxw=1):
    """The walrus build in this container rejects instructions carrying more
    than one semaphore wait ("Too many sync wait commands").  Hoist extra
    waits onto single-wait NOPs inserted just before the instruction on the
    same engine stream — the engine sequencer processes waits in program
    order, so blocking semantics are identical."""
    import concourse.mybir as mybir

    n_split = 0
    n_nops = 0
    for f in nc.m.functions:
        for b in f.blocks:
            out = []
            dirty = False
            for ins in b.instructions:
                si = getattr(ins, "sync_info", None)
                ow = list(si.on_wait) if si is not None and si.on_wait else []
                if len(ow) > maxw:
                    extra, keep = ow[:-maxw], ow[-maxw:]
                    for w in extra:
                        nop = mybir.InstNoOp(
                            name=f"{ins.name}-wsplit{n_nops}", ins=[], outs=[])
                        nop.engine = ins.engine
                        nop.sync_info = mybir.SyncInfo(on_wait=[w], on_update=[])
                        out.append(nop)
                        n_nops += 1
                    ins.sync_info = mybir.SyncInfo(
                        on_wait=keep,
                        on_update=list(si.on_update) if si.on_update else [])
                    n_split += 1
                    dirty = True
                out.append(ins)
            if dirty:
                b.instructions = out
    return n_split, n_nops


def _build_program():
    import concourse.bass as bass
    import concourse.tile as tile
    import concourse.mybir as mybir

    f32 = mybir.dt.float32
    bf16 = mybir.dt.bfloat16
    AF = mybir.ActivationFunctionType
    OP = mybir.AluOpType
    X = mybir.AxisListType.X
    ts = bass.ts

    nc = bass.Bass("TRN2", name="ntxent")
    zt = nc.dram_tensor("zt", [D, 2 * N], bf16, kind="ExternalInput")
    ztb = nc.dram_tensor("ztb", [D, RPC], bf16, kind="ExternalInput")
    pmask = nc.dram_tensor("pmask", [128, 128], f32, kind="ExternalInput")
    partial = nc.dram_tensor("partial", [1, 1], f32, kind="ExternalOutput")

    with tile.TileContext(nc) as tc:
        with (
            tc.tile_pool(name="persist", bufs=1) as persist,
            tc.tile_pool(name="io", bufs=4) as io,
            tc.tile_pool(name="work", bufs=3) as work,
            tc.tile_pool(name="scr", bufs=2) as scr,
        ):
            cpps_cm = tc.tile_pool(name="cpps", bufs=2, space="PSUM")
            cpps = cpps_cm.__enter__()
            ones_bf = persist.tile([128, 128], bf16)
            nc.vector.memset(ones_bf, 1.0)
            ones_f = persist.tile([128, 1], f32)
            nc.vector.memset(ones_f, 1.0)
            # Warm up the exp/ln activation table set while the input DMAs
            # run — the ~2.7us ACT_TABLE_LOAD otherwise lands inside the
            # first real Ln on the critical path.
            warm = persist.tile([128, 1], f32)
            nc.scalar.activation(out=warm, in_=ones_f, func=AF.Ln)
            nc.scalar.activation(out=warm, in_=warm, func=AF.Exp)
            pm = persist.tile([128, 128], f32)
            nc.sync.dma_start(pm, pmask[:, :])

            ztn = persist.tile([128, KC, 2 * N], bf16)   # normalized Z^T (rhs)
            ztnb = persist.tile([128, KC, RPC], bf16)    # normalized own block (lhsT)
            RS = persist.tile([128, MT], f32)            # rowsums
            SP = persist.tile([128, MT], f32)            # pair logits

            # ---- phase B: own block -> ztnb ----
            ztb_s = persist.tile([128, KC, RPC], bf16)
            for k in range(KC):
                nc.sync.dma_start(ztb_s[:, k, :], ztb[k * 128:(k + 1) * 128, :])
            sqb = scr.tile([128, KC, RPC], bf16, tag="sqb")
            for k in range(KC):
                nc.vector.tensor_mul(sqb[:, k, :], ztb_s[:, k, :], ztb_s[:, k, :])
            for cb in range(RPC // CB):
                qb = cpps.tile([128, CB], f32, tag="cp")
                for k in range(KC):
                    nc.tensor.matmul(qb, ones_bf, sqb[:, k, ts(cb, CB)],
                                     start=(k == 0), stop=(k == KC - 1))
                lnq = work.tile([128, CB], f32, tag="lnq")
                nc.scalar.activation(out=lnq, in_=qb, func=AF.Ln)
                rinv = work.tile([128, CB], bf16, tag="rinv")
                nc.scalar.activation(out=rinv, in_=lnq, func=AF.Exp, scale=-0.5)
                for k in range(KC):
                    nc.vector.tensor_mul(ztnb[:, k, ts(cb, CB)],
                                         ztb_s[:, k, ts(cb, CB)], rinv)

            # ---- phase C: full Z^T -> ztn, in 2048-col super-blocks ----
            SB = 4 * CB
            for sb in range(2 * N // SB):
                ztc = io.tile([128, KC, SB], bf16, tag="ztc")
                for k in range(KC):
                    nc.sync.dma_start(ztc[:, k, :],
                                      zt[k * 128:(k + 1) * 128, ts(sb, SB)])
                sqc = work.tile([128, KC, SB], bf16, tag="sqc")
                for k in range(KC):
                    nc.vector.tensor_mul(sqc[:, k, :], ztc[:, k, :], ztc[:, k, :])
                qc = cpps.tile([128, SB], f32, tag="cp")
                for ci in range(4):
                    for k in range(KC):
                        nc.tensor.matmul(qc[:, ts(ci, CB)], ones_bf,
                                         sqc[:, k, ts(ci, CB)],
                                         start=(k == 0), stop=(k == KC - 1))
                lnq = work.tile([128, SB], f32, tag="lnq")
                nc.scalar.activation(out=lnq, in_=qc, func=AF.Ln)
                rinv = work.tile([128, SB], bf16, tag="rinv")
                nc.scalar.activation(out=rinv, in_=lnq, func=AF.Exp, scale=-0.5)
                for k in range(KC):
                    nc.vector.tensor_mul(ztn[:, k, ts(sb, SB)], ztc[:, k, :], rinv)

            # ---- phase P: pair logits from diagonal sub-blocks ----
            for m in range(MT):
                ssub = cpps.tile([128, 128], f32, tag="cp")
                for k in range(KC):
                    nc.tensor.matmul(ssub, ztnb[:, k, ts(m, 128)],
                                     ztnb[:, k, ts(m, 128)],
                                     start=(k == 0), stop=(k == KC - 1))
                junk = scr.tile([128, 128], f32, tag="junk")
                nc.vector.tensor_mul(junk, ssub, pm)
                nc.vector.reduce_sum(out=SP[:, m:m + 1], in_=junk, axis=X)

            # Close the colsum PSUM pool so the main loop can use 4-bank
            # groups x2 buffers (all 8 banks).
            cpps_cm.__exit__(None, None, None)

            # ---- main: S-block matmuls + fused exp/rowsum ----
            with tc.tile_pool(name="mainps", bufs=2, space="PSUM") as mainps:
                for m in range(MT):
                    rs_m = scr.tile([128, NG], f32, tag="rsm")
                    for g in range(NG):
                        ps = mainps.tile([128, GRP * CB], f32, tag="main")
                        for ci in range(GRP):
                            cb = g * GRP + ci
                            for k in range(KC):
                                nc.tensor.matmul(ps[:, ts(ci, CB)],
                                                 ztnb[:, k, ts(m, 128)],
                                                 ztn[:, k, ts(cb, CB)],
                                                 start=(k == 0), stop=(k == KC - 1))
                        # exp in place (PSUM->PSUM): the exp'd matrix itself is
                        # discarded, only accum_out (the rowsum) is kept.
                        nc.scalar.activation(out=ps, in_=ps, func=AF.Exp,
                                             scale=2.0,
                                             accum_out=rs_m[:, g:g + 1])
                    nc.vector.reduce_sum(out=RS[:, m:m + 1], in_=rs_m, axis=X)

                # ---- final scalar ----
                DEN = persist.tile([128, MT], f32)
                nc.vector.tensor_scalar_add(DEN, RS, float(1e-8 - E2))
                LND = persist.tile([128, MT], f32)
                nc.scalar.activation(out=LND, in_=DEN, func=AF.Ln)
                LV = persist.tile([128, MT], f32)
                nc.vector.scalar_tensor_tensor(out=LV, in0=SP, scalar=-2.0,
                                               in1=LND, op0=OP.mult, op1=OP.add)
                fin = mainps.tile([1, MT], f32, tag="main")
                nc.tensor.matmul(fin, ones_f, LV, start=True, stop=True)
                tot = persist.tile([1, 1], f32)
                nc.vector.reduce_sum(out=tot, in_=fin, axis=X)
                nc.sync.dma_start(partial[:, :], tot)

    _split_multi_waits(nc)
    return nc


def _prepare_inputs(z1, z2):
    z1 = np.asarray(z1, dtype=np.float32)
    z2 = np.asarray(z2, dtype=np.float32)
    ztf = np.empty((D, 2 * N), dtype=np.float32)
    ztf[:, 0::2] = z1.T
    ztf[:, 1::2] = z2.T
    ztb16 = np.ascontiguousarray(ztf.astype(ml_dtypes.bfloat16))
    pmask = np.zeros((128, 128), dtype=np.float32)
    idx = np.arange(128)
    pmask[idx, idx ^ 1] = 1.0
    in_maps = []
    for c in range(NC):
        in_maps.append({
            "zt": ztb16,
            "ztb": np.ascontiguousarray(ztb16[:, c * RPC:(c + 1) * RPC]),
            "pmask": pmask,
        })
    return in_maps


def _run(z1, z2, trace=False):
    from concourse.bass_utils import run_bass_kernel_spmd
    if "nc" not in _prog_cache:
        _prog_cache["nc"] = _build_program()
    nc = _prog_cache["nc"]
    in_maps = _prepare_inputs(z1, z2)
    res = run_bass_kernel_spmd(nc, in_maps, core_ids=list(range(NC)), trace=trace)
    total = sum(float(r["partial"][0, 0]) for r in res.results)
    out = np.float32(total / (2 * N))
    return out, res


def kernel(z1, z2):
    out, _ = _run(z1, z2, trace=False)
    return out
